# revision 10
# baseline (speedup 1.0000x reference)
"""Trainium2 Bass kernel: CrossAttention3D (B=4, Lq=Lk=4096, D=256) on 8 NeuronCores.

Sharding: core c handles batch c//2, decoder-query half c%2 (2048 queries),
with the full encoder sequence for that batch and replicated projections.

Per-core dataflow (all matmuls in float32r = full-rate fp32-rounded):
  xdT, xeT   : PE transposes of input tiles            [d, seq]
  QT = Wq.T @ xdT  (+bq)                               [d, 2048]
  KT = Wk.T @ xeT  (+bk)                               [d, 4096]
  VeT = Wv.T @ xeT                                     [d, 4096]
  V' = (VeT).T @ Wo  with ones column appended         [4096, 257]
  per k-tile: ST = KT.T-slice @ QT -> exp(ST/16) = PT  [128k, 512q]
              O[q, 257] += PT_slice.T @ V'[ktile]      (col 256 = softmax denom)
  out = O[:, :256] / O[:, 256:] + (x_dec + bv@Wo + bo)   (host precomputes bo2)
"""
import os
import sys

import numpy as np

for _p in ("/opt/trn_rl_repo", os.path.expanduser("~/.axon_site/_ro/trn_rl_repo")):
    if os.path.isdir(_p) and _p not in sys.path:
        sys.path.insert(0, _p)

B, LQ, LK, D = 4, 4096, 4096, 256
NCORES = 8
QCHUNK = LQ // 2          # queries per core
P = 128
SCALE = 1.0 / 16.0        # 1/sqrt(D)
NQT = QCHUNK // P         # 16 query tiles per core
NKT = LK // P             # 32 key tiles per core
NQC = QCHUNK // 512       # 4 query chunks per core
NKC = LK // 512           # 8 key chunks per core

_STATE = {}


def _build():
    import concourse.tile as tile
    from concourse import bacc, mybir
    from concourse.masks import make_identity

    f32 = mybir.dt.float32
    f32r = mybir.dt.float32r
    AF = mybir.ActivationFunctionType
    ALU = mybir.AluOpType

    nc = bacc.Bacc(trn_type="TRN2")
    xd = nc.dram_tensor("xd", [QCHUNK, D], f32, kind="ExternalInput")
    xe = nc.dram_tensor("xe", [LK, D], f32, kind="ExternalInput")
    wq_d = nc.dram_tensor("wq", [D, D], f32, kind="ExternalInput")
    wk_d = nc.dram_tensor("wk", [D, D], f32, kind="ExternalInput")
    wvo_d = nc.dram_tensor("wvo", [D, D], f32, kind="ExternalInput")
    bq_d = nc.dram_tensor("bq", [D], f32, kind="ExternalInput")
    bk_d = nc.dram_tensor("bk", [D], f32, kind="ExternalInput")
    bo2_d = nc.dram_tensor("bo2", [D], f32, kind="ExternalInput")
    out = nc.dram_tensor("out", [QCHUNK, D], f32, kind="ExternalOutput")

    import concourse.bass as bass

    with tile.TileContext(nc) as tc:
        with (
            tc.tile_pool(name="singles", bufs=1) as singles,
            tc.tile_pool(name="xin", bufs=4) as xin,
            tc.tile_pool(name="tch", bufs=3) as tch,
            tc.tile_pool(name="ptp", bufs=6) as ptp,
            tc.tile_pool(name="outp", bufs=3) as outp,
            tc.tile_pool(name="recp", bufs=4) as recp,
        ):
            ident = singles.tile([P, P], f32)
            make_identity(nc, ident)

            # weights staged fp32 then rounded to fp32r
            # layout [p, ch, dout]: ch = contraction half (rows of W)
            wstage = singles.tile([P, 2, D], f32)
            wq_r = singles.tile([P, 2, D], f32r)
            wk_r = singles.tile([P, 2, D], f32r)
            wvo_r = singles.tile([P, 2, D], f32r)
            for wd, wr in ((wq_d, wq_r), (wk_d, wk_r), (wvo_d, wvo_r)):
                st = xin.tile([P, 2, D], f32, tag="wstage", bufs=2)
                for ch in range(2):
                    nc.sync.dma_start(out=st[:, ch, :], in_=wd[ch * P:(ch + 1) * P, :])
                nc.vector.tensor_copy(wr, st)
            del wstage

            bq_t = singles.tile([P, 2], f32)
            nc.sync.dma_start(out=bq_t, in_=bq_d[:].rearrange("(h p) -> p h", h=2))
            bk_t = singles.tile([P, 2], f32)
            nc.sync.dma_start(out=bk_t, in_=bk_d[:].rearrange("(h p) -> p h", h=2))

            bo2_b = singles.tile([P, D], f32)
            bo2_ap = bo2_d[:]
            nc.sync.dma_start(
                out=bo2_b,
                in_=bass.AP(tensor=bo2_ap.tensor, offset=bo2_ap.offset,
                            ap=[[0, P], [1, D]]),
            )

            # persistent per-core tensors
            xds = singles.tile([P, NQT, D], f32)      # x_dec tile + bo2 (residual)
            QT = singles.tile([P, 2, QCHUNK], f32r)
            KT = singles.tile([P, 2, LK], f32r)
            # V' with ones column (256) + zero pad column (257): fp32r matmul
            # free dims must be even, so pad 257 -> 258
            Vp = singles.tile([P, NKT, D + 2], f32r)
            ones32 = singles.tile([P, NKT, 2], f32)
            nc.vector.memset(ones32, 0.0)
            nc.vector.memset(ones32[:, :, 0:1], 1.0)

            # ---------------- phase A: decoder side -> QT ----------------
            with (
                tc.tile_pool(name="tp_ps", bufs=3, space="PSUM") as tp_ps,
                tc.tile_pool(name="pj_ps", bufs=3, space="PSUM") as pj_ps,
            ):
                for qc in range(NQC):
                    xdT_c = tch.tile([P, 2, 512], f32r, tag="tch")
                    for i in range(4):
                        ti = qc * 4 + i
                        xt = xin.tile([P, D], f32, tag="xin")
                        nc.sync.dma_start(out=xt, in_=xd[ti * P:(ti + 1) * P, :])
                        nc.vector.tensor_add(xds[:, ti, :], xt, bo2_b)
                        tp = tp_ps.tile([P, 2, P], f32, tag="tp")
                        for h in range(2):
                            nc.tensor.transpose(tp[:, h, :], xt[:, h * P:(h + 1) * P], ident)
                        nc.vector.tensor_copy(xdT_c[:, :, i * P:(i + 1) * P], tp)
                    for oh in range(2):
                        pj = pj_ps.tile([P, 512], f32, tag="pj")
                        for ch in range(2):
                            nc.tensor.matmul(pj, wq_r[:, ch, oh * P:(oh + 1) * P],
                                             xdT_c[:, ch, :],
                                             start=(ch == 0), stop=(ch == 1))
                        nc.scalar.activation(QT[:, oh, qc * 512:(qc + 1) * 512], pj,
                                             AF.Identity, bias=bq_t[:, oh:oh + 1])

                # ------------- phase B/C: encoder side -> KT, V' -------------
                for kc in range(NKC):
                    xeT_c = tch.tile([P, 2, 512], f32r, tag="tch")
                    for i in range(4):
                        ti = kc * 4 + i
                        xt = xin.tile([P, D], f32, tag="xin")
                        nc.sync.dma_start(out=xt, in_=xe[ti * P:(ti + 1) * P, :])
                        tp = tp_ps.tile([P, 2, P], f32, tag="tp")
                        for h in range(2):
                            nc.tensor.transpose(tp[:, h, :], xt[:, h * P:(h + 1) * P], ident)
                        nc.vector.tensor_copy(xeT_c[:, :, i * P:(i + 1) * P], tp)
                    for oh in range(2):
                        pj = pj_ps.tile([P, 512], f32, tag="pj")
                        for ch in range(2):
                            nc.tensor.matmul(pj, wk_r[:, ch, oh * P:(oh + 1) * P],
                                             xeT_c[:, ch, :],
                                             start=(ch == 0), stop=(ch == 1))
                        nc.scalar.activation(KT[:, oh, kc * 512:(kc + 1) * 512], pj,
                                             AF.Identity, bias=bk_t[:, oh:oh + 1])
                    for j in range(4):
                        kt_i = kc * 4 + j
                        pv = pj_ps.tile([P, 512], f32, tag="pj")
                        for ch in range(2):
                            nc.tensor.matmul(pv[:, :D], xeT_c[:, ch, j * P:(j + 1) * P],
                                             wvo_r[:, ch, :],
                                             start=(ch == 0), stop=(ch == 1))
                        nc.vector.tensor_copy(Vp[:, kt_i, 0:D], pv[:, :D])

            nc.scalar.activation(Vp[:, :, D:D + 2], ones32, AF.Copy)

            # ---------------- phase D: attention main loop ----------------
            with (
                tc.tile_pool(name="st_ps", bufs=4, space="PSUM") as st_ps,
                tc.tile_pool(name="o_ps", bufs=4, space="PSUM") as o_ps_pool,
            ):
                for qc in range(NQC):
                    o_ps = [o_ps_pool.tile([P, D + 2], f32, tag="ops",
                                           name=f"ops_{qc}_{i}")
                            for i in range(4)]
                    pts = [None] * NKT
                    for kt_i in range(NKT):
                        st = st_ps.tile([P, 512], f32, tag="st")
                        for ch in range(2):
                            nc.tensor.matmul(st, KT[:, ch, kt_i * P:(kt_i + 1) * P],
                                             QT[:, ch, qc * 512:(qc + 1) * 512],
                                             start=(ch == 0), stop=(ch == 1))
                        pt = ptp.tile([P, 512], f32r, tag="pt")
                        nc.scalar.activation(pt, st, AF.Exp, scale=SCALE)
                        pts[kt_i] = pt
                        if kt_i > 0:
                            for qs in range(4):
                                nc.tensor.matmul(o_ps[qs],
                                                 pts[kt_i - 1][:, qs * P:(qs + 1) * P],
                                                 Vp[:, kt_i - 1, :],
                                                 start=(kt_i - 1 == 0), stop=False)
                    for qs in range(4):
                        nc.tensor.matmul(o_ps[qs], pts[NKT - 1][:, qs * P:(qs + 1) * P],
                                         Vp[:, NKT - 1, :],
                                         start=False, stop=True)
                    for qs in range(4):
                        qi = qc * 4 + qs
                        rec = recp.tile([P, 1], f32, tag="rec")
                        nc.vector.reciprocal(rec, o_ps[qs][:, D:D + 1])
                        ot = outp.tile([P, D], f32, tag="ot")
                        nc.vector.scalar_tensor_tensor(
                            ot, o_ps[qs][:, 0:D], rec, xds[:, qi, :],
                            op0=ALU.mult, op1=ALU.add)
                        nc.sync.dma_start(out=out[qi * P:(qi + 1) * P, :], in_=ot)

    nc.finalize()
    return nc


def _get_nc():
    if "nc" not in _STATE:
        _STATE["nc"] = _build()
    return _STATE["nc"]


def _in_maps(x_decoder, x_encoder, Wq, bq, Wk, bk, Wv, bv, Wo, bo):
    x_decoder = np.ascontiguousarray(np.asarray(x_decoder, dtype=np.float32))
    x_encoder = np.ascontiguousarray(np.asarray(x_encoder, dtype=np.float32))
    Wq, Wk, Wv, Wo = (np.ascontiguousarray(np.asarray(w, dtype=np.float32))
                      for w in (Wq, Wk, Wv, Wo))
    bq, bk, bv, bo = (np.asarray(b, dtype=np.float32) for b in (bq, bk, bv, bo))
    bo2 = (bv.astype(np.float64) @ Wo.astype(np.float64)
           + bo.astype(np.float64)).astype(np.float32)
    Wvo = (Wv.astype(np.float64) @ Wo.astype(np.float64)).astype(np.float32)
    maps = []
    for c in range(NCORES):
        b, h = divmod(c, 2)
        maps.append({
            "xd": np.ascontiguousarray(x_decoder[b, h * QCHUNK:(h + 1) * QCHUNK]),
            "xe": x_encoder[b],
            "wq": Wq, "wk": Wk, "wvo": Wvo,
            "bq": bq, "bk": bk, "bo2": bo2,
        })
    return maps


def _assemble(results):
    out = np.empty((B, LQ, D), dtype=np.float32)
    for c in range(NCORES):
        b, h = divmod(c, 2)
        out[b, h * QCHUNK:(h + 1) * QCHUNK] = results[c]["out"]
    return out


def _get_compiled():
    """Build a reusable jitted SPMD executable (compiles once per process)."""
    if "compiled" in _STATE:
        return _STATE["compiled"]
    import jax
    import numpy as jnp_np
    from jax.sharding import Mesh, PartitionSpec
    from jax.experimental.shard_map import shard_map
    from concourse import bass2jax, mybir

    nc = _get_nc()
    bass2jax.install_neuronx_cc_hook()
    partition_name = (nc.partition_id_tensor.name
                      if nc.partition_id_tensor else None)
    in_names, out_names, out_avals, zero_outs = [], [], [], []
    for alloc in nc.m.functions[0].allocations:
        if not isinstance(alloc, mybir.MemoryLocationSet):
            continue
        name = alloc.memorylocations[0].name
        if alloc.kind == "ExternalInput":
            if name != partition_name:
                in_names.append(name)
        elif alloc.kind == "ExternalOutput":
            shape = tuple(alloc.tensor_shape)
            dtype = mybir.dt.np(alloc.dtype)
            out_names.append(name)
            out_avals.append(jax.core.ShapedArray(shape, dtype))
            zero_outs.append(np.zeros((NCORES * shape[0], *shape[1:]), dtype))
    n_params = len(in_names)
    all_names = in_names + out_names
    if partition_name is not None:
        all_names.append(partition_name)

    def _body(*args):
        operands = list(args)
        if partition_name is not None:
            operands.append(bass2jax.partition_id_tensor())
        outs = bass2jax._bass_exec_p.bind(
            *operands,
            out_avals=tuple(out_avals),
            in_names=tuple(all_names),
            out_names=tuple(out_names),
            lowering_input_output_aliases=(),
            sim_require_finite=True,
            sim_require_nnan=True,
            nc=nc,
        )
        return tuple(outs)

    devices = jax.devices()[:NCORES]
    mesh = Mesh(jnp_np.asarray(devices), ("core",))
    nio = n_params + len(out_names)
    sharded = jax.jit(
        shard_map(_body, mesh=mesh,
                  in_specs=(PartitionSpec("core"),) * nio,
                  out_specs=(PartitionSpec("core"),) * len(out_names),
                  check_rep=False),
        keep_unused=True,
    )
    _STATE["compiled"] = (sharded, in_names, out_names, out_avals, zero_outs, mesh)
    return _STATE["compiled"]


def _concat_inputs(maps, in_names):
    return [np.concatenate([maps[c][n] for c in range(NCORES)], axis=0)
            for n in in_names]


def run_maps(maps):
    sharded, in_names, out_names, out_avals, zero_outs, mesh = _get_compiled()
    concat_in = _concat_inputs(maps, in_names)
    out_arrs = sharded(*concat_in, *zero_outs)
    results = []
    for c in range(NCORES):
        results.append({
            name: np.asarray(out_arrs[i]).reshape(NCORES, *out_avals[i].shape)[c]
            for i, name in enumerate(out_names)})
    return results


def kernel(x_decoder, x_encoder, Wq, bq, Wk, bk, Wv, bv, Wo, bo):
    maps = _in_maps(x_decoder, x_encoder, Wq, bq, Wk, bk, Wv, bv, Wo, bo)
    return _assemble(run_maps(maps))


def bench(maps, iters=30):
    """Time repeated executions with device-resident inputs; returns seconds/iter."""
    import time

    import jax
    from jax.sharding import NamedSharding, PartitionSpec

    sharded, in_names, out_names, out_avals, zero_outs, mesh = _get_compiled()
    sh = NamedSharding(mesh, PartitionSpec("core"))
    dev_in = [jax.device_put(a, sh) for a in _concat_inputs(maps, in_names)]
    dev_zero = [jax.device_put(z, sh) for z in zero_outs]
    jax.block_until_ready(dev_in + dev_zero)
    out = sharded(*dev_in, *dev_zero)
    jax.block_until_ready(out)
    times = []
    for _ in range(iters):
        t0 = time.perf_counter()
        out = sharded(*dev_in, *dev_zero)
        jax.block_until_ready(out)
        times.append(time.perf_counter() - t0)
    times.sort()
    return {"min": times[0], "median": times[len(times) // 2],
            "mean": sum(times) / len(times)}


# revision 13
# speedup vs baseline: 538.1754x; 538.1754x over previous
"""Trainium2 Bass kernel: CrossAttention3D (B=4, Lq=Lk=4096, D=256) on 8 NeuronCores.

Sharding: core c handles batch c//2, decoder-query half c%2 (2048 queries),
with the full encoder sequence for that batch and replicated projections.

Per-core dataflow (all matmuls in float32r = full-rate fp32-rounded):
  xdT, xeT   : PE transposes of input tiles            [d, seq]
  QT = Wq.T @ xdT  (+bq)                               [d, 2048]
  KT = Wk.T @ xeT  (+bk)                               [d, 4096]
  VeT = Wv.T @ xeT                                     [d, 4096]
  V' = (VeT).T @ Wo  with ones column appended         [4096, 257]
  per k-tile: ST = KT.T-slice @ QT -> exp(ST/16) = PT  [128k, 512q]
              O[q, 257] += PT_slice.T @ V'[ktile]      (col 256 = softmax denom)
  out = O[:, :256] / O[:, 256:] + (x_dec + bv@Wo + bo)   (host precomputes bo2)
"""
import os
import sys

import numpy as np

for _p in ("/opt/trn_rl_repo", os.path.expanduser("~/.axon_site/_ro/trn_rl_repo")):
    if os.path.isdir(_p) and _p not in sys.path:
        sys.path.insert(0, _p)

B, LQ, LK, D = 4, 4096, 4096, 256
NCORES = 8
QCHUNK = LQ // 2          # queries per core
P = 128
SCALE = 1.0 / 16.0        # 1/sqrt(D)
NQT = QCHUNK // P         # 16 query tiles per core
NKT = LK // P             # 32 key tiles per core
NQC = QCHUNK // 512       # 4 query chunks per core
NKC = LK // 512           # 8 key chunks per core

_STATE = {}


def _build(repeat=1):
    from contextlib import ExitStack

    import concourse.tile as tile
    from concourse import bacc, mybir
    from concourse.masks import make_identity

    f32 = mybir.dt.float32
    f32r = mybir.dt.float32r
    AF = mybir.ActivationFunctionType
    ALU = mybir.AluOpType

    nc = bacc.Bacc(trn_type="TRN2")
    xd = nc.dram_tensor("xd", [QCHUNK, D], f32, kind="ExternalInput")
    xe = nc.dram_tensor("xe", [LK, D], f32, kind="ExternalInput")
    wq_d = nc.dram_tensor("wq", [D, D], f32, kind="ExternalInput")
    wk_d = nc.dram_tensor("wk", [D, D], f32, kind="ExternalInput")
    wvo_d = nc.dram_tensor("wvo", [D, D], f32, kind="ExternalInput")
    bq_d = nc.dram_tensor("bq", [D], f32, kind="ExternalInput")
    bk_d = nc.dram_tensor("bk", [D], f32, kind="ExternalInput")
    bo2_d = nc.dram_tensor("bo2", [D], f32, kind="ExternalInput")
    out = nc.dram_tensor("out", [QCHUNK, D], f32, kind="ExternalOutput")

    import concourse.bass as bass

    with tile.TileContext(nc) as tc:
        loop_ctx = ExitStack()
        if repeat > 1:
            loop_ctx.enter_context(tc.For_i(0, repeat, 1))
        with (
            tc.tile_pool(name="singles", bufs=1) as singles,
            tc.tile_pool(name="xin", bufs=4) as xin,
            tc.tile_pool(name="tch", bufs=3) as tch,
            tc.tile_pool(name="ptp", bufs=6) as ptp,
            tc.tile_pool(name="outp", bufs=3) as outp,
            tc.tile_pool(name="recp", bufs=4) as recp,
        ):
            ident = singles.tile([P, P], f32)
            make_identity(nc, ident)

            # weights staged fp32 then rounded to fp32r
            # layout [p, ch, dout]: ch = contraction half (rows of W)
            wstage = singles.tile([P, 2, D], f32)
            wq_r = singles.tile([P, 2, D], f32r)
            wk_r = singles.tile([P, 2, D], f32r)
            wvo_r = singles.tile([P, 2, D], f32r)
            for wd, wr in ((wq_d, wq_r), (wk_d, wk_r), (wvo_d, wvo_r)):
                st = xin.tile([P, 2, D], f32, tag="wstage", bufs=2)
                for ch in range(2):
                    nc.sync.dma_start(out=st[:, ch, :], in_=wd[ch * P:(ch + 1) * P, :])
                nc.vector.tensor_copy(wr, st)
            del wstage

            bq_t = singles.tile([P, 2], f32)
            nc.sync.dma_start(out=bq_t, in_=bq_d[:].rearrange("(h p) -> p h", h=2))
            bk_t = singles.tile([P, 2], f32)
            nc.sync.dma_start(out=bk_t, in_=bk_d[:].rearrange("(h p) -> p h", h=2))

            bo2_b = singles.tile([P, D], f32)
            bo2_ap = bo2_d[:]
            nc.sync.dma_start(
                out=bo2_b,
                in_=bass.AP(tensor=bo2_ap.tensor, offset=bo2_ap.offset,
                            ap=[[0, P], [1, D]]),
            )

            # persistent per-core tensors
            xds = singles.tile([P, NQT, D], f32)      # x_dec tile + bo2 (residual)
            QT = singles.tile([P, 2, QCHUNK], f32r)
            KT = singles.tile([P, 2, LK], f32r)
            # V' with ones column (256) + zero pad column (257): fp32r matmul
            # free dims must be even, so pad 257 -> 258
            Vp = singles.tile([P, NKT, D + 2], f32r)
            ones32 = singles.tile([P, NKT, 2], f32)
            nc.vector.memset(ones32, 0.0)
            nc.vector.memset(ones32[:, :, 0:1], 1.0)

            # ---------------- phase A: decoder side -> QT ----------------
            with (
                tc.tile_pool(name="tp_ps", bufs=3, space="PSUM") as tp_ps,
                tc.tile_pool(name="pj_ps", bufs=3, space="PSUM") as pj_ps,
            ):
                for qc in range(NQC):
                    xdT_c = tch.tile([P, 2, 512], f32r, tag="tch")
                    for i in range(4):
                        ti = qc * 4 + i
                        xt = xin.tile([P, D], f32, tag="xin")
                        nc.sync.dma_start(out=xt, in_=xd[ti * P:(ti + 1) * P, :])
                        nc.vector.tensor_add(xds[:, ti, :], xt, bo2_b)
                        tp = tp_ps.tile([P, 2, P], f32, tag="tp")
                        for h in range(2):
                            nc.tensor.transpose(tp[:, h, :], xt[:, h * P:(h + 1) * P], ident)
                        nc.vector.tensor_copy(xdT_c[:, :, i * P:(i + 1) * P], tp)
                    for oh in range(2):
                        pj = pj_ps.tile([P, 512], f32, tag="pj")
                        for ch in range(2):
                            nc.tensor.matmul(pj, wq_r[:, ch, oh * P:(oh + 1) * P],
                                             xdT_c[:, ch, :],
                                             start=(ch == 0), stop=(ch == 1))
                        nc.scalar.activation(QT[:, oh, qc * 512:(qc + 1) * 512], pj,
                                             AF.Identity, bias=bq_t[:, oh:oh + 1])

                # ------------- phase B/C: encoder side -> KT, V' -------------
                for kc in range(NKC):
                    xeT_c = tch.tile([P, 2, 512], f32r, tag="tch")
                    for i in range(4):
                        ti = kc * 4 + i
                        xt = xin.tile([P, D], f32, tag="xin")
                        nc.sync.dma_start(out=xt, in_=xe[ti * P:(ti + 1) * P, :])
                        tp = tp_ps.tile([P, 2, P], f32, tag="tp")
                        for h in range(2):
                            nc.tensor.transpose(tp[:, h, :], xt[:, h * P:(h + 1) * P], ident)
                        nc.vector.tensor_copy(xeT_c[:, :, i * P:(i + 1) * P], tp)
                    for oh in range(2):
                        pj = pj_ps.tile([P, 512], f32, tag="pj")
                        for ch in range(2):
                            nc.tensor.matmul(pj, wk_r[:, ch, oh * P:(oh + 1) * P],
                                             xeT_c[:, ch, :],
                                             start=(ch == 0), stop=(ch == 1))
                        nc.scalar.activation(KT[:, oh, kc * 512:(kc + 1) * 512], pj,
                                             AF.Identity, bias=bk_t[:, oh:oh + 1])
                    for j in range(4):
                        kt_i = kc * 4 + j
                        pv = pj_ps.tile([P, 512], f32, tag="pj")
                        for ch in range(2):
                            nc.tensor.matmul(pv[:, :D], xeT_c[:, ch, j * P:(j + 1) * P],
                                             wvo_r[:, ch, :],
                                             start=(ch == 0), stop=(ch == 1))
                        nc.vector.tensor_copy(Vp[:, kt_i, 0:D], pv[:, :D])

            nc.scalar.activation(Vp[:, :, D:D + 2], ones32, AF.Copy)

            # ---------------- phase D: attention main loop ----------------
            with (
                tc.tile_pool(name="st_ps", bufs=4, space="PSUM") as st_ps,
                tc.tile_pool(name="o_ps", bufs=4, space="PSUM") as o_ps_pool,
            ):
                for qc in range(NQC):
                    o_ps = [o_ps_pool.tile([P, D + 2], f32, tag="ops",
                                           name=f"ops_{qc}_{i}")
                            for i in range(4)]
                    pts = [None] * NKT
                    for kt_i in range(NKT):
                        st = st_ps.tile([P, 512], f32, tag="st")
                        for ch in range(2):
                            nc.tensor.matmul(st, KT[:, ch, kt_i * P:(kt_i + 1) * P],
                                             QT[:, ch, qc * 512:(qc + 1) * 512],
                                             start=(ch == 0), stop=(ch == 1))
                        pt = ptp.tile([P, 512], f32r, tag="pt")
                        nc.scalar.activation(pt, st, AF.Exp, scale=SCALE)
                        pts[kt_i] = pt
                        if kt_i > 0:
                            for qs in range(4):
                                nc.tensor.matmul(o_ps[qs],
                                                 pts[kt_i - 1][:, qs * P:(qs + 1) * P],
                                                 Vp[:, kt_i - 1, :],
                                                 start=(kt_i - 1 == 0), stop=False)
                    for qs in range(4):
                        nc.tensor.matmul(o_ps[qs], pts[NKT - 1][:, qs * P:(qs + 1) * P],
                                         Vp[:, NKT - 1, :],
                                         start=False, stop=True)
                    for qs in range(4):
                        qi = qc * 4 + qs
                        rec = recp.tile([P, 1], f32, tag="rec")
                        nc.vector.reciprocal(rec, o_ps[qs][:, D:D + 1])
                        ot = outp.tile([P, D], f32, tag="ot")
                        nc.vector.scalar_tensor_tensor(
                            ot, o_ps[qs][:, 0:D], rec, xds[:, qi, :],
                            op0=ALU.mult, op1=ALU.add)
                        nc.sync.dma_start(out=out[qi * P:(qi + 1) * P, :], in_=ot)

        loop_ctx.close()

    nc.finalize()
    return nc


def _get_nc(repeat=1):
    key = f"nc{repeat}"
    if key not in _STATE:
        _STATE[key] = _build(repeat)
    return _STATE[key]


def _in_maps(x_decoder, x_encoder, Wq, bq, Wk, bk, Wv, bv, Wo, bo):
    x_decoder = np.ascontiguousarray(np.asarray(x_decoder, dtype=np.float32))
    x_encoder = np.ascontiguousarray(np.asarray(x_encoder, dtype=np.float32))
    Wq, Wk, Wv, Wo = (np.ascontiguousarray(np.asarray(w, dtype=np.float32))
                      for w in (Wq, Wk, Wv, Wo))
    bq, bk, bv, bo = (np.asarray(b, dtype=np.float32) for b in (bq, bk, bv, bo))
    bo2 = (bv.astype(np.float64) @ Wo.astype(np.float64)
           + bo.astype(np.float64)).astype(np.float32)
    Wvo = (Wv.astype(np.float64) @ Wo.astype(np.float64)).astype(np.float32)
    maps = []
    for c in range(NCORES):
        b, h = divmod(c, 2)
        maps.append({
            "xd": np.ascontiguousarray(x_decoder[b, h * QCHUNK:(h + 1) * QCHUNK]),
            "xe": x_encoder[b],
            "wq": Wq, "wk": Wk, "wvo": Wvo,
            "bq": bq, "bk": bk, "bo2": bo2,
        })
    return maps


def _assemble(results):
    out = np.empty((B, LQ, D), dtype=np.float32)
    for c in range(NCORES):
        b, h = divmod(c, 2)
        out[b, h * QCHUNK:(h + 1) * QCHUNK] = results[c]["out"]
    return out


def _get_compiled(repeat=1):
    """Build a reusable jitted SPMD executable (compiles once per process)."""
    ckey = f"compiled{repeat}"
    if ckey in _STATE:
        return _STATE[ckey]
    import jax
    import numpy as jnp_np
    from jax.sharding import Mesh, PartitionSpec
    from jax.experimental.shard_map import shard_map
    from concourse import bass2jax, mybir

    nc = _get_nc(repeat)
    bass2jax.install_neuronx_cc_hook()
    partition_name = (nc.partition_id_tensor.name
                      if nc.partition_id_tensor else None)
    in_names, out_names, out_avals, zero_outs = [], [], [], []
    for alloc in nc.m.functions[0].allocations:
        if not isinstance(alloc, mybir.MemoryLocationSet):
            continue
        name = alloc.memorylocations[0].name
        if alloc.kind == "ExternalInput":
            if name != partition_name:
                in_names.append(name)
        elif alloc.kind == "ExternalOutput":
            shape = tuple(alloc.tensor_shape)
            dtype = mybir.dt.np(alloc.dtype)
            out_names.append(name)
            out_avals.append(jax.core.ShapedArray(shape, dtype))
            zero_outs.append(np.zeros((NCORES * shape[0], *shape[1:]), dtype))
    n_params = len(in_names)
    all_names = in_names + out_names
    if partition_name is not None:
        all_names.append(partition_name)

    def _body(*args):
        operands = list(args)
        if partition_name is not None:
            operands.append(bass2jax.partition_id_tensor())
        outs = bass2jax._bass_exec_p.bind(
            *operands,
            out_avals=tuple(out_avals),
            in_names=tuple(all_names),
            out_names=tuple(out_names),
            lowering_input_output_aliases=(),
            sim_require_finite=True,
            sim_require_nnan=True,
            nc=nc,
        )
        return tuple(outs)

    devices = jax.devices()[:NCORES]
    mesh = Mesh(jnp_np.asarray(devices), ("core",))
    nio = n_params + len(out_names)
    sharded = jax.jit(
        shard_map(_body, mesh=mesh,
                  in_specs=(PartitionSpec("core"),) * nio,
                  out_specs=(PartitionSpec("core"),) * len(out_names),
                  check_rep=False),
        keep_unused=True,
    )
    _STATE[ckey] = (sharded, in_names, out_names, out_avals, zero_outs, mesh)
    return _STATE[ckey]


def _concat_inputs(maps, in_names):
    return [np.concatenate([maps[c][n] for c in range(NCORES)], axis=0)
            for n in in_names]


def run_maps(maps):
    sharded, in_names, out_names, out_avals, zero_outs, mesh = _get_compiled()
    concat_in = _concat_inputs(maps, in_names)
    out_arrs = sharded(*concat_in, *zero_outs)
    results = []
    for c in range(NCORES):
        results.append({
            name: np.asarray(out_arrs[i]).reshape(NCORES, *out_avals[i].shape)[c]
            for i, name in enumerate(out_names)})
    return results


def kernel(x_decoder, x_encoder, Wq, bq, Wk, bk, Wv, bv, Wo, bo):
    maps = _in_maps(x_decoder, x_encoder, Wq, bq, Wk, bk, Wv, bv, Wo, bo)
    return _assemble(run_maps(maps))


def bench(maps, iters=30, repeat=1):
    """Time repeated executions with device-resident inputs; returns seconds/iter."""
    import time

    import jax
    from jax.sharding import NamedSharding, PartitionSpec

    sharded, in_names, out_names, out_avals, zero_outs, mesh = _get_compiled(repeat)
    sh = NamedSharding(mesh, PartitionSpec("core"))
    dev_in = [jax.device_put(a, sh) for a in _concat_inputs(maps, in_names)]
    dev_zero = [jax.device_put(z, sh) for z in zero_outs]
    jax.block_until_ready(dev_in + dev_zero)
    out = sharded(*dev_in, *dev_zero)
    jax.block_until_ready(out)
    times = []
    for _ in range(iters):
        t0 = time.perf_counter()
        out = sharded(*dev_in, *dev_zero)
        jax.block_until_ready(out)
        times.append(time.perf_counter() - t0)
    times.sort()
    return {"min": times[0], "median": times[len(times) // 2],
            "mean": sum(times) / len(times)}
